# revision 1
# baseline (speedup 1.0000x reference)
"""Trainium2 Bass kernel for a dense causal-attention transformer block.

Reference computation (fp32, B=2, S=2048, D=2048, H=16, HD=128):
    qkv = x @ Wqkv ; q,k,v split per head
    scores = (q @ k^T) * HD**-0.5, causal mask, softmax
    o = softmax(scores) @ v ; out = o @ Wo

Sharding: tensor-parallel over heads (4 groups of 4 heads) x data-parallel
over batch (2) = 8 cores. Each core computes a partial output projection
(its 512 o-channels x Wo rows); the host sums the 4 partials per batch.

Device layout tricks:
  - All matmul inputs are bf16 (4x faster PE than fp32); PSUM accum fp32.
  - qT/kT are produced channels-on-partitions so score tiles come out
    TRANSPOSED [keys=128, queries=512]; softmax sum is then a matmul with
    an all-ones lhsT (no cross-partition reduce, no transposes anywhere).
  - No max-subtraction in softmax: scores ~ N(0,1), exp is safe in fp32,
    and masked entries are multiplied by 0 after exp.
  - HD**-0.5 scaling folded into Wq on the host.
"""

import numpy as np
import ml_dtypes

BF16 = ml_dtypes.bfloat16

B = 2
S = 2048
D = 2048
H = 16
HD = 128
P = 128
G = 4            # TP groups (heads per group = 4)
NH = H // G      # heads per core = 4
CH = NH * HD     # o-channels per core = 512
NJ = S // 512    # 4 S-chunks of 512
KK = D // P      # 16 contraction tiles
ST = S // P      # 16 sequence row-tiles

_prog = None


def _build():
    """Build (once) the single-core Bass/Tile program shared by all 8 cores."""
    global _prog
    if _prog is not None:
        return _prog

    import concourse.bass as bass
    import concourse.tile as tile
    from concourse import bacc, mybir

    f32 = mybir.dt.float32
    bf16 = mybir.dt.bfloat16
    EXP = mybir.ActivationFunctionType.Exp

    nc = bacc.Bacc("TRN2", target_bir_lowering=False, debug=False)

    # DRAM I/O, pre-packed on host so every DMA is contiguous per partition.
    # x:  [p, nj, kk, q]  = xT chunk layout (x[b].T tiled)
    # wq/wk: [p, mi, kk, m] (column-sharded Wqkv, q part prescaled by HD^-.5)
    # wv: [p, kk, n]      (rhs layout)
    # wo: [p, h, ncol, n] (row-sharded Wo)
    # masks: [k, j, q]    binary causal masks for the 4 diagonal positions
    # out: [p, si, col]   partial output (fp32)
    x_d = nc.dram_tensor("x", (P, NJ, KK, 512), bf16, kind="ExternalInput")
    wq_d = nc.dram_tensor("wq", (P, NH, KK, P), bf16, kind="ExternalInput")
    wk_d = nc.dram_tensor("wk", (P, NH, KK, P), bf16, kind="ExternalInput")
    wv_d = nc.dram_tensor("wv", (P, KK, CH), bf16, kind="ExternalInput")
    wo_d = nc.dram_tensor("wo", (P, NH, NJ, 512), bf16, kind="ExternalInput")
    mask_d = nc.dram_tensor("masks", (P, NH, 512), bf16, kind="ExternalInput")
    out_d = nc.dram_tensor("out", (P, ST, D), f32, kind="ExternalOutput")

    with tile.TileContext(nc) as tc:
        with (
            tc.tile_pool(name="persist", bufs=1) as pp,
            tc.tile_pool(name="psumA", bufs=4, space="PSUM") as psA,
            tc.tile_pool(name="psumB", bufs=2, space="PSUM") as psB,
        ):
            wq_sb = pp.tile([P, NH, KK, P], bf16, name="wq_sb", tag="wq")
            wk_sb = pp.tile([P, NH, KK, P], bf16, name="wk_sb", tag="wk")
            # wv (phase 1) and wo (phase 3) share one 16KB slot
            wv_sb = pp.tile([P, KK, CH], bf16, name="wv_sb", tag="wvwo")
            q_sb = pp.tile([P, NH, S], bf16, name="q_sb", tag="q")
            k_sb = pp.tile([P, NH, S], bf16, name="k_sb", tag="k")
            v_sb = pp.tile([P, ST, CH], bf16, name="v_sb", tag="v")
            o_sb = pp.tile([P, NH, S], bf16, name="o_sb", tag="o")
            mask_sb = pp.tile([P, NH, 512], bf16, name="mask_sb", tag="mask")
            ones_sb = pp.tile([P, P], bf16, name="ones_sb", tag="ones")
            zbias = pp.tile([P, 1], f32, name="zbias", tag="zbias")

            nc.sync.dma_start(wq_sb[:], wq_d[:])
            nc.sync.dma_start(wk_sb[:], wk_d[:])
            nc.sync.dma_start(wv_sb[:], wv_d[:])
            nc.sync.dma_start(mask_sb[:], mask_d[:])
            nc.gpsimd.memset(ones_sb[:], 1.0)
            nc.gpsimd.memset(zbias[:], 0.0)

            # ---------------- Phase 1: QKV projections ----------------
            with tc.tile_pool(name="xpool", bufs=2) as xpool:
                for nj in range(NJ):
                    xc = xpool.tile([P, KK, 512], bf16, name=f"xc{nj}", tag="xc")
                    nc.sync.dma_start(xc[:], x_d[:, nj])
                    # qT, kT: [CH, S] channel-major (per head = 128 partitions)
                    for w_sb, dst in ((wq_sb, q_sb), (wk_sb, k_sb)):
                        for mi in range(NH):
                            acc = psA.tile([P, 512], f32, name=f"qk{nj}_{mi}",
                                           tag="accA")
                            for kk in range(KK):
                                nc.tensor.matmul(
                                    acc[:], w_sb[:, mi, kk, :], xc[:, kk, :],
                                    start=(kk == 0), stop=(kk == KK - 1))
                            nc.scalar.copy(
                                out=dst[:, mi, nj * 512:(nj + 1) * 512],
                                in_=acc[:])
                    # v: [S, CH] row-major (keys on partitions)
                    for si in range(4):
                        sg = 4 * nj + si
                        acc = psA.tile([P, CH], f32, name=f"v{sg}", tag="accA")
                        for kk in range(KK):
                            nc.tensor.matmul(
                                acc[:], xc[:, kk, si * P:(si + 1) * P],
                                wv_sb[:, kk, :],
                                start=(kk == 0), stop=(kk == KK - 1))
                        nc.scalar.copy(out=v_sb[:, sg, :], in_=acc[:])

            # wo reuses wv's slot (Tile serializes the DMA after last wv read)
            wo_sb = pp.tile([P, NH, NJ, 512], bf16, name="wo_sb", tag="wvwo")
            nc.sync.dma_start(wo_sb[:], wo_d[:])

            # ---------- Phase 2+3: attention + output projection ----------
            with (
                tc.tile_pool(name="apool", bufs=20) as apool,
                tc.tile_pool(name="rpool", bufs=3) as rpool,
                tc.tile_pool(name="ostage", bufs=4) as ostage,
            ):
                for qc in range(NJ):          # query chunk of 512
                    qs, qe = qc * 512, (qc + 1) * 512
                    ktmax = 4 * qc + 4        # causal: key tiles 0..ktmax-1
                    for h in range(NH):
                        # scoresT tiles [keys=128, queries=512] -> exp -> a
                        a_tiles = []
                        for kt in range(ktmax):
                            st = psA.tile([P, 512], f32,
                                          name=f"st{qc}_{h}_{kt}", tag="accA")
                            nc.tensor.matmul(
                                st[:], k_sb[:, h, kt * P:(kt + 1) * P],
                                q_sb[:, h, qs:qe], start=True, stop=True)
                            a_t = apool.tile([P, 512], bf16,
                                             name=f"a{qc}_{h}_{kt}", tag="a")
                            nc.scalar.activation(a_t[:], st[:], EXP,
                                                 bias=zbias[:])
                            if kt >= 4 * qc:  # diagonal tile: apply causal 0/1
                                nc.vector.tensor_mul(
                                    out=a_t[:], in0=a_t[:],
                                    in1=mask_sb[:, kt - 4 * qc, :])
                            a_tiles.append(a_t)
                        # oT accumulation: [HD, 512] += v_kt^T-contract a_kt
                        po = psB.tile([P, 512], f32, name=f"po{qc}_{h}",
                                      tag="po")
                        for kt in range(ktmax):
                            nc.tensor.matmul(
                                po[:], v_sb[:, kt, h * HD:(h + 1) * HD],
                                a_tiles[kt][:],
                                start=(kt == 0), stop=(kt == ktmax - 1))
                        # normalizer: ones^T @ a = column sums, replicated
                        pn = psB.tile([P, 512], f32, name=f"pn{qc}_{h}",
                                      tag="pn")
                        for kt in range(ktmax):
                            nc.tensor.matmul(
                                pn[:], ones_sb[:], a_tiles[kt][:],
                                start=(kt == 0), stop=(kt == ktmax - 1))
                        rec = rpool.tile([P, 512], f32, name=f"rc{qc}_{h}",
                                         tag="rec")
                        nc.vector.reciprocal(rec[:], pn[:])
                        nc.vector.tensor_mul(out=o_sb[:, h, qs:qe],
                                             in0=po[:], in1=rec[:])

                    # output projection for the 4 row-tiles of this q chunk
                    for si in range(4 * qc, 4 * qc + 4):
                        for ncol in range(NJ):
                            acc = psA.tile([P, 512], f32,
                                           name=f"pr{si}_{ncol}", tag="accA")
                            for h in range(NH):
                                nc.tensor.matmul(
                                    acc[:], o_sb[:, h, si * P:(si + 1) * P],
                                    wo_sb[:, h, ncol, :],
                                    start=(h == 0), stop=(h == NH - 1))
                            stg = ostage.tile([P, 512], f32,
                                              name=f"os{si}_{ncol}", tag="os")
                            nc.vector.tensor_copy(out=stg[:], in_=acc[:])
                            nc.sync.dma_start(
                                out_d[:, si, ncol * 512:(ncol + 1) * 512],
                                stg[:])

    nc.compile()
    _prog = nc
    return nc


def _pack_inputs(x, Wqkv, Wo):
    """Host-side shard + pack into the per-core DMA-friendly layouts."""
    scale = np.float32(HD) ** np.float32(-0.5)
    masks = np.zeros((P, NH, 512), dtype=BF16)
    k_idx = np.arange(P)[:, None]
    q_idx = np.arange(512)[None, :]
    for j in range(NH):
        masks[:, j, :] = (P * j + k_idx <= q_idx).astype(BF16)

    in_maps = []
    for c in range(8):
        b, g = divmod(c, G)
        xb = np.asarray(x[b], dtype=np.float32)
        # xT packed: [p, nj, kk, q] with xT[128*kk+p, 512*nj+q] = xb[q', d']
        xp = np.ascontiguousarray(
            xb.astype(BF16).reshape(NJ, 512, KK, P).transpose(3, 0, 2, 1))
        wq = (np.asarray(Wqkv[:, CH * g:CH * (g + 1)], np.float32) * scale)
        wk = np.asarray(Wqkv[:, D + CH * g:D + CH * (g + 1)], np.float32)
        wv = np.asarray(Wqkv[:, 2 * D + CH * g:2 * D + CH * (g + 1)],
                        np.float32)
        wo = np.asarray(Wo[CH * g:CH * (g + 1), :], np.float32)
        wq_p = np.ascontiguousarray(
            wq.astype(BF16).reshape(KK, P, NH, P).transpose(1, 2, 0, 3))
        wk_p = np.ascontiguousarray(
            wk.astype(BF16).reshape(KK, P, NH, P).transpose(1, 2, 0, 3))
        wv_p = np.ascontiguousarray(
            wv.astype(BF16).reshape(KK, P, CH).transpose(1, 0, 2))
        wo_p = np.ascontiguousarray(
            wo.astype(BF16).reshape(NH, P, NJ, 512).transpose(1, 0, 2, 3))
        in_maps.append({
            "x": xp, "wq": wq_p, "wk": wk_p, "wv": wv_p, "wo": wo_p,
            "masks": masks,
        })
    return in_maps


def _unpack_outputs(results):
    """Sum the 4 TP partials per batch and restore [B, S, D]."""
    out = np.zeros((B, S, D), dtype=np.float32)
    for c, res in enumerate(results):
        b = c // G
        part = np.asarray(res["out"])           # [p, si, col]
        out[b] += part.transpose(1, 0, 2).reshape(S, D)
    return out


def kernel(x, Wqkv, Wo, _trace=False, _trace_kwargs=None):
    from concourse import bass_utils

    nc = _build()
    in_maps = _pack_inputs(x, Wqkv, Wo)
    res = bass_utils.run_bass_kernel_spmd(
        nc, in_maps, core_ids=list(range(8)), trace=_trace,
        **(_trace_kwargs or {}))
    out = _unpack_outputs(res.results)
    if _trace:
        kernel.last_result = res
    return out


# revision 3
# speedup vs baseline: 18.0874x; 18.0874x over previous
"""Trainium2 Bass kernel for a dense causal-attention transformer block.

Reference computation (fp32, B=2, S=2048, D=2048, H=16, HD=128):
    qkv = x @ Wqkv ; q,k,v split per head
    scores = (q @ k^T) * HD**-0.5, causal mask, softmax
    o = softmax(scores) @ v ; out = o @ Wo

Sharding: tensor-parallel over heads (4 groups of 4 heads) x data-parallel
over batch (2) = 8 cores. Each core computes a partial output projection
(its 512 o-channels x Wo rows); the host sums the 4 partials per batch.

Device layout tricks:
  - All matmul inputs are bf16 (4x faster PE than fp32); PSUM accum fp32.
  - qT/kT are produced channels-on-partitions so score tiles come out
    TRANSPOSED [keys=128, queries=512]; softmax sum is then a matmul with
    an all-ones lhsT (no cross-partition reduce, no transposes anywhere).
  - No max-subtraction in softmax: scores ~ N(0,1), exp is safe in fp32,
    and masked entries are multiplied by 0 after exp.
  - HD**-0.5 scaling folded into Wq on the host.
"""

import numpy as np
import ml_dtypes

BF16 = ml_dtypes.bfloat16

B = 2
S = 2048
D = 2048
H = 16
HD = 128
P = 128
G = 4            # TP groups (heads per group = 4)
NH = H // G      # heads per core = 4
CH = NH * HD     # o-channels per core = 512
NJ = S // 512    # 4 S-chunks of 512
KK = D // P      # 16 contraction tiles
ST = S // P      # 16 sequence row-tiles

_progs = {}


def _build(repeat=1):
    """Build (once) the single-core Bass/Tile program shared by all 8 cores.

    repeat>1 executes the whole computation that many times inside one NEFF
    (used only for overhead-free timing via T(xN)-T(x1) differencing).
    """
    if repeat in _progs:
        return _progs[repeat]

    import concourse.tile as tile
    from concourse import bacc, mybir

    f32 = mybir.dt.float32
    bf16 = mybir.dt.bfloat16
    EXP = mybir.ActivationFunctionType.Exp

    nc = bacc.Bacc("TRN2", target_bir_lowering=False, debug=False)

    # DRAM I/O, pre-packed on host so every DMA is contiguous per partition.
    # x:  [p, nj, kk, q]  = xT chunk layout (x[b].T tiled)
    # wq/wk: [p, mi, kk, m] (column-sharded Wqkv, q part prescaled by HD^-.5)
    # wv: [p, kk, n]      (rhs layout)
    # wo: [p, h, ncol, n] (row-sharded Wo)
    # masks: [k, j, q]    binary causal masks for the 4 diagonal positions
    # out: [p, si, col]   partial output (fp32)
    x_d = nc.dram_tensor("x", (P, NJ, KK, 512), bf16, kind="ExternalInput")
    wq_d = nc.dram_tensor("wq", (P, NH, KK, P), bf16, kind="ExternalInput")
    wk_d = nc.dram_tensor("wk", (P, NH, KK, P), bf16, kind="ExternalInput")
    wv_d = nc.dram_tensor("wv", (P, KK, CH), bf16, kind="ExternalInput")
    wo_d = nc.dram_tensor("wo", (P, NH, NJ, 512), bf16, kind="ExternalInput")
    mask_d = nc.dram_tensor("masks", (P, NH, 512), bf16, kind="ExternalInput")
    out_d = nc.dram_tensor("out", (P, ST, D), f32, kind="ExternalOutput")

    with tile.TileContext(nc) as tc:
        with (
            tc.tile_pool(name="persist", bufs=1) as pp,
            tc.tile_pool(name="psumA", bufs=4, space="PSUM") as psA,
            tc.tile_pool(name="psumB", bufs=2, space="PSUM") as psB,
        ):
            for rep in range(repeat):
                _emit_once(nc, tc, tile, mybir, pp, psA, psB,
                           x_d, wq_d, wk_d, wv_d, wo_d, mask_d, out_d,
                           f32, bf16, EXP, rep)

    nc.compile()
    _progs[repeat] = nc
    return nc


def _emit_once(nc, tc, tile, mybir, pp, psA, psB,
               x_d, wq_d, wk_d, wv_d, wo_d, mask_d, out_d,
               f32, bf16, EXP, rep):
    r = f"r{rep}_"
    wq_sb = pp.tile([P, NH, KK, P], bf16, name=r + "wq_sb", tag="wq")
    wk_sb = pp.tile([P, NH, KK, P], bf16, name=r + "wk_sb", tag="wk")
    # wv (phase 1) and wo (phase 3) share one 16KB slot
    wv_sb = pp.tile([P, KK, CH], bf16, name=r + "wv_sb", tag="wvwo")
    q_sb = pp.tile([P, NH, S], bf16, name=r + "q_sb", tag="q")
    k_sb = pp.tile([P, NH, S], bf16, name=r + "k_sb", tag="k")
    v_sb = pp.tile([P, ST, CH], bf16, name=r + "v_sb", tag="v")
    o_sb = pp.tile([P, NH, S], bf16, name=r + "o_sb", tag="o")
    mask_sb = pp.tile([P, NH, 512], bf16, name=r + "mask_sb", tag="mask")
    ones_sb = pp.tile([P, P], bf16, name=r + "ones_sb", tag="ones")
    zbias = pp.tile([P, 1], f32, name=r + "zbias", tag="zbias")

    nc.sync.dma_start(wq_sb[:], wq_d[:])
    nc.sync.dma_start(wk_sb[:], wk_d[:])
    nc.sync.dma_start(wv_sb[:], wv_d[:])
    nc.sync.dma_start(mask_sb[:], mask_d[:])
    nc.gpsimd.memset(ones_sb[:], 1.0)
    nc.gpsimd.memset(zbias[:], 0.0)

    # ---------------- Phase 1: QKV projections ----------------
    with tc.tile_pool(name=r + "xpool", bufs=2) as xpool:
        for nj in range(NJ):
            xc = xpool.tile([P, KK, 512], bf16, name=f"{r}xc{nj}", tag="xc")
            nc.sync.dma_start(xc[:], x_d[:, nj])
            # qT, kT: [CH, S] channel-major (per head = 128 partitions)
            for w_sb, dst in ((wq_sb, q_sb), (wk_sb, k_sb)):
                for mi in range(NH):
                    acc = psA.tile([P, 512], f32, name=f"{r}qk{nj}_{mi}",
                                   tag="accA")
                    for kk in range(KK):
                        nc.tensor.matmul(
                            acc[:], w_sb[:, mi, kk, :], xc[:, kk, :],
                            start=(kk == 0), stop=(kk == KK - 1))
                    nc.scalar.copy(
                        out=dst[:, mi, nj * 512:(nj + 1) * 512],
                        in_=acc[:])
            # v: [S, CH] row-major (keys on partitions)
            for si in range(4):
                sg = 4 * nj + si
                acc = psA.tile([P, CH], f32, name=f"{r}v{sg}", tag="accA")
                for kk in range(KK):
                    nc.tensor.matmul(
                        acc[:], xc[:, kk, si * P:(si + 1) * P],
                        wv_sb[:, kk, :],
                        start=(kk == 0), stop=(kk == KK - 1))
                nc.scalar.copy(out=v_sb[:, sg, :], in_=acc[:])

    # wo reuses wv's slot (Tile serializes the DMA after last wv read)
    wo_sb = pp.tile([P, NH, NJ, 512], bf16, name=r + "wo_sb", tag="wvwo")
    nc.sync.dma_start(wo_sb[:], wo_d[:])

    # ---------- Phase 2+3: attention + output projection ----------
    with (
        tc.tile_pool(name=r + "apool", bufs=20) as apool,
        tc.tile_pool(name=r + "rpool", bufs=3) as rpool,
        tc.tile_pool(name=r + "ostage", bufs=4) as ostage,
    ):
        for qc in range(NJ):          # query chunk of 512
            qs, qe = qc * 512, (qc + 1) * 512
            ktmax = 4 * qc + 4        # causal: key tiles 0..ktmax-1
            for h in range(NH):
                # scoresT tiles [keys=128, queries=512] -> exp -> a
                a_tiles = []
                for kt in range(ktmax):
                    st = psA.tile([P, 512], f32,
                                  name=f"{r}st{qc}_{h}_{kt}", tag="accA")
                    nc.tensor.matmul(
                        st[:], k_sb[:, h, kt * P:(kt + 1) * P],
                        q_sb[:, h, qs:qe], start=True, stop=True)
                    a_t = apool.tile([P, 512], bf16,
                                     name=f"{r}a{qc}_{h}_{kt}", tag="a")
                    nc.scalar.activation(a_t[:], st[:], EXP,
                                         bias=zbias[:])
                    if kt >= 4 * qc:  # diagonal tile: apply causal 0/1
                        nc.vector.tensor_mul(
                            out=a_t[:], in0=a_t[:],
                            in1=mask_sb[:, kt - 4 * qc, :])
                    a_tiles.append(a_t)
                # oT accumulation: [HD, 512] += v_kt^T-contract a_kt
                po = psB.tile([P, 512], f32, name=f"{r}po{qc}_{h}",
                              tag="po")
                for kt in range(ktmax):
                    nc.tensor.matmul(
                        po[:], v_sb[:, kt, h * HD:(h + 1) * HD],
                        a_tiles[kt][:],
                        start=(kt == 0), stop=(kt == ktmax - 1))
                # normalizer: ones^T @ a = column sums, replicated
                pn = psB.tile([P, 512], f32, name=f"{r}pn{qc}_{h}",
                              tag="pn")
                for kt in range(ktmax):
                    nc.tensor.matmul(
                        pn[:], ones_sb[:], a_tiles[kt][:],
                        start=(kt == 0), stop=(kt == ktmax - 1))
                rec = rpool.tile([P, 512], f32, name=f"{r}rc{qc}_{h}",
                                 tag="rec")
                nc.vector.reciprocal(rec[:], pn[:])
                nc.vector.tensor_mul(out=o_sb[:, h, qs:qe],
                                     in0=po[:], in1=rec[:])

            # output projection for the 4 row-tiles of this q chunk
            for si in range(4 * qc, 4 * qc + 4):
                for ncol in range(NJ):
                    acc = psA.tile([P, 512], f32,
                                   name=f"{r}pr{si}_{ncol}", tag="accA")
                    for h in range(NH):
                        nc.tensor.matmul(
                            acc[:], o_sb[:, h, si * P:(si + 1) * P],
                            wo_sb[:, h, ncol, :],
                            start=(h == 0), stop=(h == NH - 1))
                    stg = ostage.tile([P, 512], f32,
                                      name=f"{r}os{si}_{ncol}", tag="os")
                    nc.vector.tensor_copy(out=stg[:], in_=acc[:])
                    nc.sync.dma_start(
                        out_d[:, si, ncol * 512:(ncol + 1) * 512],
                        stg[:])


def _pack_inputs(x, Wqkv, Wo):
    """Host-side shard + pack into the per-core DMA-friendly layouts."""
    scale = np.float32(HD) ** np.float32(-0.5)
    masks = np.zeros((P, NH, 512), dtype=BF16)
    k_idx = np.arange(P)[:, None]
    q_idx = np.arange(512)[None, :]
    for j in range(NH):
        masks[:, j, :] = (P * j + k_idx <= q_idx).astype(BF16)

    in_maps = []
    for c in range(8):
        b, g = divmod(c, G)
        xb = np.asarray(x[b], dtype=np.float32)
        # xT packed: [p, nj, kk, q] with xT[128*kk+p, 512*nj+q] = xb[q', d']
        xp = np.ascontiguousarray(
            xb.astype(BF16).reshape(NJ, 512, KK, P).transpose(3, 0, 2, 1))
        wq = (np.asarray(Wqkv[:, CH * g:CH * (g + 1)], np.float32) * scale)
        wk = np.asarray(Wqkv[:, D + CH * g:D + CH * (g + 1)], np.float32)
        wv = np.asarray(Wqkv[:, 2 * D + CH * g:2 * D + CH * (g + 1)],
                        np.float32)
        wo = np.asarray(Wo[CH * g:CH * (g + 1), :], np.float32)
        wq_p = np.ascontiguousarray(
            wq.astype(BF16).reshape(KK, P, NH, P).transpose(1, 2, 0, 3))
        wk_p = np.ascontiguousarray(
            wk.astype(BF16).reshape(KK, P, NH, P).transpose(1, 2, 0, 3))
        wv_p = np.ascontiguousarray(
            wv.astype(BF16).reshape(KK, P, CH).transpose(1, 0, 2))
        wo_p = np.ascontiguousarray(
            wo.astype(BF16).reshape(NH, P, NJ, 512).transpose(1, 0, 2, 3))
        in_maps.append({
            "x": xp, "wq": wq_p, "wk": wk_p, "wv": wv_p, "wo": wo_p,
            "masks": masks,
        })
    return in_maps


def _unpack_outputs(results):
    """Sum the 4 TP partials per batch and restore [B, S, D]."""
    out = np.zeros((B, S, D), dtype=np.float32)
    for c, res in enumerate(results):
        b = c // G
        part = np.asarray(res["out"])           # [p, si, col]
        out[b] += part.transpose(1, 0, 2).reshape(S, D)
    return out


def kernel(x, Wqkv, Wo, _trace=False, _trace_kwargs=None):
    from concourse import bass_utils

    nc = _build()
    in_maps = _pack_inputs(x, Wqkv, Wo)
    res = bass_utils.run_bass_kernel_spmd(
        nc, in_maps, core_ids=list(range(8)), trace=_trace,
        **(_trace_kwargs or {}))
    out = _unpack_outputs(res.results)
    if _trace:
        kernel.last_result = res
    return out
